# revision 12
# baseline (speedup 1.0000x reference)
"""Trainium2 Bass kernel for Block (2D overlapping patch extraction).

x: [4, 8, 512, 512] f32 -> out: [4, 8, 3969, 16, 16]
block 16x16, stride 8x8, 'valid' -> nbh = nbw = 63.

Sharding: data-parallel over the 32 (batch*channel) images; 4 images per
NeuronCore across 8 cores. No cross-core communication.

Per-core plan (images c in [0,4), block rows i in [0,63)):
 - i is processed in two chunks (32 + 31) so that (c, i) fits in <=128 SBUF
   partitions.
 - Load: partition (c*nI + ii) holds input rows [8*(i0+ii), 8*(i0+ii)+16) of
   image c -- a 32 KiB contiguous DRAM run per partition (rows are read ~2x
   due to the stride-8 overlap; this buys fully-contiguous DMA descriptors).
 - Rearrange on-chip: 4 vector-engine copies (one per (a, b) quadrant of the
   2x2 cell decomposition of a 16x16 block) gather the patch layout
   out[j, a*8+h, b*8+w] = in[8a+h, 8(j+b)+w] within each partition.
 - Store: partition (c*nI + ii) now holds out[c, i*63:(i+1)*63, :, :]
   verbatim -- a 63 KiB contiguous DRAM run per partition.
"""

import numpy as np

NCORES = 8
CH_PER_CORE = 4
H = W = 512
BH = BW = 16
SH = SW = 8
NB = 63          # blocks per axis
ROW = W          # elements per image row
IMG = H * W      # elements per image
OUT_BLK = BH * BW            # 256 elements per block
OUT_ROWCH = NB * OUT_BLK     # 16128 elements per block-row
OUT_IMG = NB * NB * OUT_BLK  # elements per output image

_CACHE = {}


def _build_nc():
    import concourse.bass as bass
    import concourse.bacc as bacc
    import concourse.mybir as mybir
    from concourse import tile

    nc = bacc.Bacc(
        "TRN2", target_bir_lowering=False, debug=False, num_devices=NCORES
    )
    xs = nc.dram_tensor(
        "xs", [CH_PER_CORE, H, W], mybir.dt.float32, kind="ExternalInput"
    )
    out = nc.dram_tensor(
        "out",
        [CH_PER_CORE, NB * NB, BH, BW],
        mybir.dt.float32,
        kind="ExternalOutput",
    )
    out_rows = out.rearrange("c (i j) h w -> c i (j h w)", i=NB)

    GRP = SH * ROW  # 4096 elements: one 8-row group
    J_GROUPS = ((0, 16), (16, 16), (32, 16), (48, 15))
    # Store-queue rotation: all three DMA queues carry stores.  Stores sit at
    # the END of the gpsimd FIFO (after every load has issued), so they can't
    # head-of-line-block the loads.
    STORE_ENG = {
        (0, 0): "gpsimd", (0, 1): "sync", (0, 2): "scalar", (0, 3): "sync",
        (31, 0): "gpsimd", (31, 1): "scalar", (31, 2): "sync", (31, 3): "scalar",
    }
    with tile.TileContext(nc) as tc:
        with (
            tc.tile_pool(name="lp", bufs=2) as lp,
            tc.tile_pool(name="op", bufs=2) as op,
        ):
            # Both chunks are 32 block-rows (overlapping at i=31, which is
            # stored twice with identical bytes): the DMA engine spray only
            # engages all 16 engines when the AP outer count is a multiple
            # of 16 -- a 31-row chunk ran on 1-2 engines.
            for i0 in (0, 31):
                nI = 32
                P = CH_PER_CORE * nI  # 128
                # Partition p = ii*4 + c (row-group-major): the DRAM AP's
                # outermost dim count drives the SDMA engine spray -- outer
                # count 4 pins a DMA to 4 engines (~67GB/s), outer count that
                # is a multiple of 16 engages all 16 (~400GB/s).
                # L[p=(ii,c)] = rows [8*(i0+ii), 8*(i0+ii)+16) of image c, as
                # two 16KB-per-partition loads (a=0/a=1 halves; >16KB
                # descriptors run at half the per-engine rate).  Rows are read
                # ~2x from HBM; an in-SBUF halo is impossible because engine
                # APs cannot start at an unaligned partition (+4 shift).
                L = lp.tile([128, 2 * GRP], mybir.dt.float32, name=f"L{i0}", tag="L")
                for a in (0, 1):
                    nc.gpsimd.dma_start(
                        out=L[:, a * GRP : (a + 1) * GRP],
                        in_=bass.AP(
                            xs,
                            (i0 + a) * GRP,
                            [[GRP, nI], [IMG, CH_PER_CORE], [1, GRP]],
                        ),
                    )

                O = op.tile([128, OUT_ROWCH], mybir.dt.float32, name=f"O{i0}", tag="O")
                O_r = O.rearrange(
                    "p (j A h B w) -> p A B j h w", j=NB, A=2, h=SH, B=2, w=SW
                )
                L_r = L.rearrange("p (A h col) -> p A h col", A=2, h=SH, col=ROW)

                # Copies grouped by j-group so each store can fire as soon as
                # its quarter of O is ready.  out[p][j, a*8+h, b*8+w] =
                # in[8*(i0+ii+a)+h, 8*(j+b)+w] = L[p][a, h, 8*(j+b)+w].
                for gi, (j0, njg) in enumerate(J_GROUPS):
                    for a in (0, 1):
                        for b in (0, 1):
                            c0 = SW * (j0 + b)
                            nc.vector.tensor_copy(
                                out=O_r[:, a, b, j0 : j0 + njg],
                                in_=L_r[:, a, :, c0 : c0 + SW * njg].rearrange(
                                    "p h (j w) -> p j h w", w=SW
                                ),
                            )
                    eng = getattr(nc, STORE_ENG[(i0, gi)])
                    eng.dma_start(
                        out=bass.AP(
                            out,
                            i0 * OUT_ROWCH + j0 * OUT_BLK,
                            [
                                [OUT_ROWCH, nI],
                                [OUT_IMG, CH_PER_CORE],
                                [1, njg * OUT_BLK],
                            ],
                        ),
                        in_=O[:P, j0 * OUT_BLK : (j0 + njg) * OUT_BLK],
                    )
    nc.compile()
    return nc


def get_nc():
    if "nc" not in _CACHE:
        _CACHE["nc"] = _build_nc()
    return _CACHE["nc"]


def _enable_jax_compile_cache():
    try:
        import jax

        jax.config.update("jax_compilation_cache_dir", "/tmp/jax_neff_cache")
        jax.config.update("jax_persistent_cache_min_entry_size_bytes", -1)
        jax.config.update("jax_persistent_cache_min_compile_time_secs", 0.0)
    except Exception:
        pass


def run_spmd(in_maps, **kwargs):
    from concourse.bass_utils import run_bass_kernel_spmd

    _enable_jax_compile_cache()
    return run_bass_kernel_spmd(
        get_nc(), in_maps, core_ids=list(range(NCORES)), **kwargs
    )


def make_in_maps(x: np.ndarray):
    xs = np.asarray(x, dtype=np.float32).reshape(-1, H, W)
    return [
        {"xs": np.ascontiguousarray(xs[c * CH_PER_CORE : (c + 1) * CH_PER_CORE])}
        for c in range(NCORES)
    ]


def assemble(results, batch_shape):
    outs = np.stack([r["out"] for r in results])  # [8, 4, 3969, 16, 16]
    return outs.reshape(*batch_shape, NB * NB, BH, BW)


def kernel(**inputs) -> np.ndarray:
    x = np.asarray(inputs["x"])
    res = run_spmd(make_in_maps(x))
    return assemble(res.results, x.shape[:2])


# revision 15
# speedup vs baseline: 1.1806x; 1.1806x over previous
"""Trainium2 Bass kernel for Block (2D overlapping patch extraction).

x: [4, 8, 512, 512] f32 -> out: [4, 8, 3969, 16, 16]
block 16x16, stride 8x8, 'valid' -> nbh = nbw = 63.

Sharding: data-parallel over the 32 (batch*channel) images; 4 images per
NeuronCore across 8 cores. No cross-core communication.

Per-core plan (images c in [0,4), block rows i in [0,63)):
 - i is processed in two chunks (32 + 31) so that (c, i) fits in <=128 SBUF
   partitions.
 - Load: partition (c*nI + ii) holds input rows [8*(i0+ii), 8*(i0+ii)+16) of
   image c -- a 32 KiB contiguous DRAM run per partition (rows are read ~2x
   due to the stride-8 overlap; this buys fully-contiguous DMA descriptors).
 - Rearrange on-chip: 4 vector-engine copies (one per (a, b) quadrant of the
   2x2 cell decomposition of a 16x16 block) gather the patch layout
   out[j, a*8+h, b*8+w] = in[8a+h, 8(j+b)+w] within each partition.
 - Store: partition (c*nI + ii) now holds out[c, i*63:(i+1)*63, :, :]
   verbatim -- a 63 KiB contiguous DRAM run per partition.
"""

import numpy as np

NCORES = 8
CH_PER_CORE = 4
H = W = 512
BH = BW = 16
SH = SW = 8
NB = 63          # blocks per axis
ROW = W          # elements per image row
IMG = H * W      # elements per image
OUT_BLK = BH * BW            # 256 elements per block
OUT_ROWCH = NB * OUT_BLK     # 16128 elements per block-row
OUT_IMG = NB * NB * OUT_BLK  # elements per output image

_CACHE = {}


def _build_nc():
    import concourse.bass as bass
    import concourse.bacc as bacc
    import concourse.mybir as mybir
    from concourse import tile

    nc = bacc.Bacc(
        "TRN2", target_bir_lowering=False, debug=False, num_devices=NCORES
    )
    xs = nc.dram_tensor(
        "xs", [CH_PER_CORE, H, W], mybir.dt.float32, kind="ExternalInput"
    )
    out = nc.dram_tensor(
        "out",
        [CH_PER_CORE, NB * NB, BH, BW],
        mybir.dt.float32,
        kind="ExternalOutput",
    )
    out_rows = out.rearrange("c (i j) h w -> c i (j h w)", i=NB)

    GRP = SH * ROW  # 4096 elements: one 8-row group
    J_GROUPS = ((0, 16), (16, 16), (32, 16), (48, 15))
    # Store-queue rotation: all three DMA queues carry stores.  Stores sit at
    # the END of the gpsimd FIFO (after every load has issued), so they can't
    # head-of-line-block the loads.
    STORE_ENG = {
        (0, 0): "sync", (0, 1): "scalar", (0, 2): "sync", (0, 3): "scalar",
        (31, 0): "scalar", (31, 1): "sync", (31, 2): "scalar", (31, 3): "sync",
    }
    with tile.TileContext(nc) as tc:
        with (
            tc.tile_pool(name="lp", bufs=2) as lp,
            tc.tile_pool(name="op", bufs=2) as op,
        ):
            # Both chunks are 32 block-rows (overlapping at i=31, which is
            # stored twice with identical bytes): the DMA engine spray only
            # engages all 16 engines when the AP outer count is a multiple
            # of 16 -- a 31-row chunk ran on 1-2 engines.
            nI = 32
            P = CH_PER_CORE * nI  # 128
            # All loads emitted first, and loads are the only gpsimd-queue
            # DMAs: a store in that FIFO would wait on copies and
            # head-of-line-block the next chunk's loads behind it.
            Ls = {}
            for i0 in (0, 31):
                # Partition p = ii*4 + c (row-group-major): the DRAM AP's
                # outermost dim count drives the SDMA engine spray -- outer
                # count 4 pins a DMA to 4 engines (~67GB/s), outer count that
                # is a multiple of 16 engages all 16 (~400GB/s).
                # L[p=(ii,c)] = rows [8*(i0+ii), 8*(i0+ii)+16) of image c, as
                # two 16KB-per-partition loads (a=0/a=1 halves; >16KB
                # descriptors run at half the per-engine rate).  Rows are read
                # ~2x from HBM; an in-SBUF halo is impossible because engine
                # APs cannot start at an unaligned partition (+4 shift).
                L = lp.tile(
                    [128, 2 * GRP], mybir.dt.float32, name=f"L{i0}", tag=f"L{i0}", bufs=1
                )
                Ls[i0] = L
                for a in (0, 1):
                    nc.gpsimd.dma_start(
                        out=L[:, a * GRP : (a + 1) * GRP],
                        in_=bass.AP(
                            xs,
                            (i0 + a) * GRP,
                            [[GRP, nI], [IMG, CH_PER_CORE], [1, GRP]],
                        ),
                    )

            for i0 in (0, 31):
                L = Ls[i0]
                O = op.tile([128, OUT_ROWCH], mybir.dt.float32, name=f"O{i0}", tag="O")
                O_r = O.rearrange(
                    "p (j A h B w) -> p A B j h w", j=NB, A=2, h=SH, B=2, w=SW
                )
                L_r = L.rearrange("p (A h col) -> p A h col", A=2, h=SH, col=ROW)

                # Copies grouped by j-group so each store can fire as soon as
                # its quarter of O is ready.  out[p][j, a*8+h, b*8+w] =
                # in[8*(i0+ii+a)+h, 8*(j+b)+w] = L[p][a, h, 8*(j+b)+w].
                for gi, (j0, njg) in enumerate(J_GROUPS):
                    for a in (0, 1):
                        for b in (0, 1):
                            c0 = SW * (j0 + b)
                            nc.vector.tensor_copy(
                                out=O_r[:, a, b, j0 : j0 + njg],
                                in_=L_r[:, a, :, c0 : c0 + SW * njg].rearrange(
                                    "p h (j w) -> p j h w", w=SW
                                ),
                            )
                    eng = getattr(nc, STORE_ENG[(i0, gi)])
                    eng.dma_start(
                        out=bass.AP(
                            out,
                            i0 * OUT_ROWCH + j0 * OUT_BLK,
                            [
                                [OUT_ROWCH, nI],
                                [OUT_IMG, CH_PER_CORE],
                                [1, njg * OUT_BLK],
                            ],
                        ),
                        in_=O[:P, j0 * OUT_BLK : (j0 + njg) * OUT_BLK],
                    )
    nc.compile()
    return nc


def get_nc():
    if "nc" not in _CACHE:
        _CACHE["nc"] = _build_nc()
    return _CACHE["nc"]


def _enable_jax_compile_cache():
    try:
        import jax

        jax.config.update("jax_compilation_cache_dir", "/tmp/jax_neff_cache")
        jax.config.update("jax_persistent_cache_min_entry_size_bytes", -1)
        jax.config.update("jax_persistent_cache_min_compile_time_secs", 0.0)
    except Exception:
        pass


def run_spmd(in_maps, **kwargs):
    from concourse.bass_utils import run_bass_kernel_spmd

    _enable_jax_compile_cache()
    return run_bass_kernel_spmd(
        get_nc(), in_maps, core_ids=list(range(NCORES)), **kwargs
    )


def make_in_maps(x: np.ndarray):
    xs = np.asarray(x, dtype=np.float32).reshape(-1, H, W)
    return [
        {"xs": np.ascontiguousarray(xs[c * CH_PER_CORE : (c + 1) * CH_PER_CORE])}
        for c in range(NCORES)
    ]


def assemble(results, batch_shape):
    outs = np.stack([r["out"] for r in results])  # [8, 4, 3969, 16, 16]
    return outs.reshape(*batch_shape, NB * NB, BH, BW)


def kernel(**inputs) -> np.ndarray:
    x = np.asarray(inputs["x"])
    res = run_spmd(make_in_maps(x))
    return assemble(res.results, x.shape[:2])
